# revision 6
# baseline (speedup 1.0000x reference)
"""Box filter (radius 8, window 17, zero-padded edges) over dims 2,3 of a
[8, 32, 512, 512] f32 tensor, on 8 Trainium2 NeuronCores.

v2 (fp16 device pipeline, no-halo tiling):
  - The harness tolerance is rel_err < 2e-2; computing on-device in fp16
    (input quantized on host, output upconverted on host) halves HBM traffic
    (64 -> 32 MiB per core), and the fp32 baseline was 91% DMA-busy.
    Expected numeric error ~1e-3 (scan state is fp32 internally; only I/O
    quantization matters).
  - Column (free-dim) filter: ONE fused DVE `tensor_tensor_scan` per channel
    over a [128, 4*537] buffer holding four 128-row blocks, each padded
    [17 zeros | 512 data | 8 zeros]; the 25 zeros between blocks flush the
    recurrence  state[t] = (x[t] + state[t-1]) - x[t-17],  so scan position
    537*b + c + 8 holds the window centered at image column c of block b.
  - Row (partition-dim) filter: blocks carry NO halo rows (exactly rows
    128b..128b+127 on partitions).  Each 128-row output tile t accumulates
    in PSUM:  main banded matmul (block t)  +  corner matmuls for the <=8
    boundary rows contributed by blocks t-1 / t+1.
  - Loads are one 512 KB DMA per channel (no halo re-read), stores one
    512 KB DMA per channel.

Sharding: data-parallel over batch (dim 0) -> 8 cores, one batch each.
"""

import os
import sys

import numpy as np

for _p in ("/opt/trn_rl_repo", "/root/.axon_site/_ro/trn_rl_repo"):
    if os.path.isdir(_p) and _p not in sys.path:
        sys.path.append(_p)

import concourse.bass as bass
import concourse.tile as tile
from concourse import bacc, mybir
from concourse.bass_utils import run_bass_kernel_spmd

R = 8
PADF = 2 * R + 1  # front zero pad per block (window width)
PADB = R          # back zero pad per block
H = W = 512
CH = 32
NCORES = 8
NB = 4            # 128-row blocks per channel
XW = PADF + W + PADB          # 537 block stride in the scan buffer
XALL = NB * XW                # 2148
UBW = XALL - PADF             # 2131 scan output width

# Number of channels whose scan runs on GPSIMD instead of DVE (tunable;
# DVE is the bottleneck engine at ~4.5us/channel).  Spread evenly across
# the channel sequence so the in-order PE/ACT queues never wait long on a
# slow GPSIMD scan.
N_GPSIMD = int(os.environ.get("BOX_GSCAN", "0"))
GSCAN_SET = (
    {2 + round(i * CH / N_GPSIMD) for i in range(N_GPSIMD)}
    if N_GPSIMD else set()
)

_CACHE = {}


def _banded():
    k = np.arange(128)[:, None]
    m = np.arange(128)[None, :]
    # main: block t rows -> tile t outputs, |k - m| <= 8
    bm = (np.abs(k - m) <= R).astype(np.float16)
    # prev corner: block t-1 row k (image 128t-128+k) -> output m, m <= 7:
    # |m + 128 - k| <= 8  <=>  k >= m + 120.  lhsT [128, 8].
    cp = ((k >= m + 120) & (m <= 7)).astype(np.float16)
    # next corner: block t+1 row k (image 128t+128+k) -> output m >= 120:
    # |m - 128 - k| <= 8  <=>  k <= m - 120.  lhsT [128, 128] (cols <120 zero).
    cn = ((k <= m - 120) & (m >= 120)).astype(np.float16)
    return bm, cp, cn


def _build_program():
    if "nc" in _CACHE:
        return _CACHE["nc"]
    nc = bacc.Bacc(debug=False)
    f16 = mybir.dt.float16
    f32 = mybir.dt.float32
    x = nc.dram_tensor("x", [CH, H, W], f16, kind="ExternalInput")
    z = nc.dram_tensor("z", [CH, H, W], f16, kind="ExternalOutput")
    bm = nc.dram_tensor("bm", [128, 128], f16, kind="ExternalInput")
    cp = nc.dram_tensor("cp", [128, 128], f16, kind="ExternalInput")
    cn = nc.dram_tensor("cn", [128, 128], f16, kind="ExternalInput")
    xap, zap = x.ap(), z.ap()

    NBIG = 6   # xa ring
    NUB = 6    # scan-out ring
    NOG = 4    # output ring

    with tile.TileContext(nc) as tc:
        with (
            tc.tile_pool(name="consts", bufs=1) as cpool,
            tc.tile_pool(name="psum", bufs=2, space="PSUM") as ppool,
        ):
            bmt = cpool.tile([128, 128], f16)
            cpt = cpool.tile([128, 128], f16)
            cnt = cpool.tile([128, 128], f16)

            xas = [
                nc.alloc_sbuf_tensor(f"xa{i}", [128, XALL], f16).ap()
                for i in range(NBIG)
            ]

            def _zero_pads(xb):
                # loads only ever write the data columns, so zeroing just
                # the pad columns once is enough for the whole kernel
                front = bass.AP(
                    tensor=xb.tensor, offset=xb.offset,
                    ap=[[XALL, 128], [XW, NB], [1, PADF]],
                )
                back = bass.AP(
                    tensor=xb.tensor, offset=xb.offset + PADF + W,
                    ap=[[XALL, 128], [XW, NB], [1, PADB]],
                )
                nc.gpsimd.memset(front, 0.0)
                nc.gpsimd.memset(back, 0.0)

            _zero_pads(xas[0])
            ubs = [
                nc.alloc_sbuf_tensor(f"ub{i}", [128, UBW], f16).ap()
                for i in range(NUB)
            ]
            ogs = [
                nc.alloc_sbuf_tensor(f"og{i}", [128, NB, W], f16).ap()
                for i in range(NOG)
            ]
            # gpsimd timing probes: self-contained scratch chain, runs on
            # the otherwise-idle Pool engine mid-kernel (no pipeline deps)
            prb = nc.alloc_sbuf_tensor("prb", [128, XALL], f16).ap()
            nc.gpsimd.memset(prb[:, :], 0.0)

            for c in range(CH):
                xa = xas[c % NBIG]
                ub = ubs[c % NUB]
                og = ogs[c % NOG]

                # one 512 KB load: (p, b, col) <- x[c, 128b + p, col]
                src = xap[c, :, :].rearrange("(b p) w -> p b w", p=128)
                dst = bass.AP(
                    tensor=xa.tensor,
                    offset=xa.offset + PADF,
                    ap=[[XALL, 128], [XW, NB], [1, W]],
                )
                nc.sync.dma_start(dst, src)
                if c == 0:
                    # consts + remaining ring zeroing overlap channel 0's load
                    nc.sync.dma_start(bmt[:], bm.ap()[:, :])
                    nc.sync.dma_start(cpt[:], cp.ap()[:, :])
                    nc.sync.dma_start(cnt[:], cn.ap()[:, :])
                    for xb in xas[1:]:
                        _zero_pads(xb)

                # one scan covers all 4 blocks (recurrence flushes in the
                # 25-zero inter-block gaps).  out[t] = window of 17 ending
                # at data0 position t.
                eng = nc.gpsimd if c in GSCAN_SET else nc.vector
                eng.tensor_tensor_scan(
                    out=ub[:, 0:UBW],
                    data0=xa[:, PADF:XALL],
                    data1=xa[:, 0:UBW],
                    initial=0.0,
                    op0=mybir.AluOpType.add,
                    op1=mybir.AluOpType.subtract,
                )

                ps = ppool.tile([128, NB, W], f32)
                last = c == CH - 1
                if c == 16:
                    for _ in range(4):
                        nc.gpsimd.tensor_add(
                            out=prb[:, 0:XALL - 16],
                            in0=prb[:, 0:XALL - 16],
                            in1=prb[:, 16:XALL],
                        )
                    nc.gpsimd.tensor_copy(
                        out=prb[:, 0:XALL - 16], in_=prb[:, 16:XALL]
                    )

                for t in range(NB):
                    rhs = ub[:, t * XW + R:t * XW + R + W]
                    nc.tensor.matmul(
                        ps[:, t, :], bmt[0:128, 0:128], rhs,
                        start=True, stop=False,
                    )
                    if t > 0:
                        rhs_p = ub[:, (t - 1) * XW + R:(t - 1) * XW + R + W]
                        nc.tensor.matmul(
                            ps[:, t, :], cpt[0:128, 0:128], rhs_p,
                            start=False, stop=(t == NB - 1),
                        )
                    if t < NB - 1:
                        rhs_n = ub[:, (t + 1) * XW + R:(t + 1) * XW + R + W]
                        nc.tensor.matmul(
                            ps[:, t, :], cnt[0:128, 0:128], rhs_n,
                            start=False, stop=True,
                        )
                    if last:
                        # fine-grained tail: per-tile copy + store
                        nc.scalar.copy(og[:, t, :], ps[:, t, :])
                        nc.scalar.dma_start(
                            zap[c, t * 128:(t + 1) * 128, :], og[:, t, :]
                        )
                if not last:
                    # one big PSUM->SBUF copy (amortizes the PSUM access
                    # latency over all 4 banks), one 512 KB store
                    nc.scalar.copy(og[:, :, :], ps[:, :, :])
                    nc.scalar.dma_start(
                        zap[c, :, :].rearrange("(t p) w -> p t w", p=128),
                        og[:, :, :],
                    )

    nc.compile()
    _CACHE["nc"] = nc
    return nc


def kernel(tensor: np.ndarray) -> np.ndarray:
    tensor = np.asarray(tensor)
    assert tensor.shape == (NCORES, CH, H, W)
    x16 = tensor.astype(np.float16)
    bm, cp, cn = _banded()
    nc = _build_program()
    in_maps = [
        {"x": x16[i], "bm": bm, "cp": cp, "cn": cn} for i in range(NCORES)
    ]
    res = run_bass_kernel_spmd(nc, in_maps, core_ids=list(range(NCORES)))
    out = np.stack([res.results[i]["z"] for i in range(NCORES)], axis=0)
    return out.astype(np.float32)
